# revision 29
# baseline (speedup 1.0000x reference)
"""DiceLoss Trainium2 kernel (8-core data-parallel SPMD, bf16).

Math (equivalent to the reference):
  softmax over channels is monotone, so pred_cls = argmax_c pred[:, c].
  p_counts[c] = #{pixels: argmax == c}
  t_counts[c] = #{pixels: target == c}
  overlap[c]  = #{pixels: argmax == c and target == c}
  dice = 2*overlap / (p_counts + t_counts + 1);  loss = 1 - dice.sum()/(N*C)

Sharding: batch dim across the 8 NeuronCores (one image per core). Inputs
are shipped as bf16 (halves HBM traffic and doubles DVE tensor_tensor
throughput); the only deviation from the f32 reference is argmax flips on
near-ties after bf16 rounding (~0.4% of pixels), worth ~2.5e-5 relative
error on the final scalar. All counting below is exact.

Device algorithm per core (pred shard [19, 512*512], t shard, 2 pixel
chunks, staged sub-DMAs so compute starts while the chunk streams in):
  pass A (DVE):  per-pixel max m over the 19 channels, pairwise max tree.
  pass B (DVE+ACT): d_c = x_c - m on DVE (2x-mode tensor_tensor), then the
      Activation engine computes the *exact* indicator
      eq_c = Relu(2^60 * d_c + 1)  (d==0 -> 1; any nonzero bf16 d has
      |2^60*d| >> 1 so Relu clamps to 0), with fused accum_out giving the
      per-partition p_count partial for free.
  pass C (DVE): scalar_tensor_tensor (t == c) * eq_c with fused accum ->
      overlap partials.
  t_counts (ACT): cumulative ReLU moments W_j = sum_i Relu(t_i - (j-1));
      the host recovers counts by second differencing. Exact: per-partition
      sums stay under 2^24.
Partials are [128, col] f32 integers -> DMA'd out, summed exactly on the
host across partitions/cores, and combined into the final f32 scalar.

Measured on trn2: ~113 us HW exec (f32 memory roofline for this problem is
~56 us/core; DVE and ACT both run ~86% busy - the kernel is compute-bound
on the two elementwise-capable engines; GPSIMD tensor ops are rejected by
this walrus pipeline and the PE has no role in argmax/counting).
"""

import sys

for _p in ("/opt/trn_rl_repo",):
    if _p not in sys.path:
        sys.path.insert(0, _p)

from contextlib import ExitStack

import numpy as np
from ml_dtypes import bfloat16

import concourse.bass as bass
import concourse.bacc as bacc
import concourse.mybir as mybir
import concourse.tile as tile
from concourse.bass_utils import run_bass_kernel_spmd

# Problem constants (hardcoded; kernel.py must be self-contained).
N_CORES = 8
C = 19
H = W = 512
PIX = H * W  # pixels per core = 262144
P = 128  # SBUF partitions
FTOT = PIX // P  # 2048 free elems per partition
NCHUNK = 2
F = FTOT // NCHUNK  # 512 pixels per partition per chunk

FP32 = mybir.dt.float32
BF16 = mybir.dt.bfloat16
Alu = mybir.AluOpType
Act = mybir.ActivationFunctionType

# Output accumulator layout: [128, NCOL]
#   p_counts: col  (c*NCHUNK + k)            for c in 0..18, k chunk
#   overlap:  col  PC_COLS + (c*NCHUNK + k)
#   t_counts: col  2*PC_COLS + c
PC_COLS = C * NCHUNK
N_TC_DVE = 8
NCOL = 2 * PC_COLS + C + N_TC_DVE



def build_program():
    nc = bacc.Bacc("TRN2", target_bir_lowering=False, debug=False,
                   num_devices=N_CORES)
    pred = nc.dram_tensor("pred", [C, PIX], BF16, kind="ExternalInput").ap()
    tin = nc.dram_tensor("t", [PIX], BF16, kind="ExternalInput").ap()
    out = nc.dram_tensor("out", [P, NCOL], FP32, kind="ExternalOutput").ap()
    negm_d = nc.dram_tensor("negm_scratch", [NCHUNK, P, F], BF16).ap()

    # DRAM views: chunk k, partition p, class c, free f
    pred_r = pred.rearrange("c (k p f) -> k p c f", k=NCHUNK, p=P, f=F)
    t_r = tin.rearrange("(k p f) -> p k f", k=NCHUNK, p=P, f=F)

    with tile.TileContext(nc) as tc, ExitStack() as ctx:
        xpool = ctx.enter_context(tc.tile_pool(name="x", bufs=2))
        mpool = ctx.enter_context(tc.tile_pool(name="m", bufs=2))
        jpool = ctx.enter_context(tc.tile_pool(name="junk", bufs=2))
        tpool = ctx.enter_context(tc.tile_pool(name="t", bufs=1))
        apool = ctx.enter_context(tc.tile_pool(name="acc", bufs=1))
        spool = ctx.enter_context(tc.tile_pool(name="scr", bufs=2))

        N_TC_DVE = 8
        acc = apool.tile([P, PC_COLS + N_TC_DVE], FP32)  # DVE accums (overlap + 8 W-moments)
        acc_t = apool.tile([P, C + PC_COLS], FP32)  # ACT-written accums
        nc.gpsimd.memset(acc_t[:, :C], 0.0)  # W slots j>=11 live in acc

        # t resident for the whole kernel: [128, (k f)]
        t_all = tpool.tile([P, NCHUNK * F], BF16)
        nc.sync.dma_start(
            t_all[:].rearrange("p (k f) -> p k f", k=NCHUNK, f=F), t_r)

        # per-class bias columns: cbias[:, j] = -(j - 1) = 1, 0, -1, ..., -17
        cbias_i = apool.tile([P, C], mybir.dt.int32)
        nc.gpsimd.iota(cbias_i[:], [[1, C]], channel_multiplier=0)
        cbias = apool.tile([P, C], FP32)
        nc.scalar.activation(cbias[:], cbias_i[:], Act.Copy, scale=-1.0,
                             bias=1.0)

        # ---- t_counts on ACT via cumulative ReLU moments ----
        # W_c = sum_i Relu(t_i - c) for c = -1..17 (W_18 = 0); host recovers
        # n_c = (W_{c-1}-W_c) - (W_c-W_{c+1}) exactly (integer partial sums
        # stay under 2^24 per partition).
        for j in range(C):
            if j >= C - N_TC_DVE:
                # W_j = sum max(t, j-1) on DVE (4x-mode tensor_scalar);
                # first differences of sum-max give cumulative counts, same
                # second-difference decode as the Relu moments.
                uj = spool.tile([P, NCHUNK * F], BF16, tag="dvu")
                nc.vector.tensor_scalar(
                    uj[:], t_all[:], float(j - 1), 0.0, Alu.max, Alu.add,
                    accum_out=acc[:, PC_COLS + (j - (C - N_TC_DVE)):
                                  PC_COLS + (j - (C - N_TC_DVE)) + 1])
            else:
                u = spool.tile([P, NCHUNK * F], BF16, tag="actu")
                nc.scalar.activation(u[:], t_all[:], Act.Relu,
                                     bias=cbias[:, j:j + 1],
                                     accum_out=acc_t[:, j:j + 1])

        # ---- main per-chunk passes ----
        for k in range(NCHUNK):
            x = xpool.tile([P, C, F], BF16)
            # staged sub-DMAs: pass A can start on classes 0-1 while the
            # rest of the chunk is still in flight
            for lo_c, hi_c in ((0, 2), (2, 8), (8, 14), (14, C)):
                nc.sync.dma_start(x[:, lo_c:hi_c, :], pred_r[k, :, lo_c:hi_c, :])
            tk = t_all[:, k * F:(k + 1) * F]

            # pass A: pairwise max tree (independent ops per level avoid
            # the RAW pipeline stalls a serial chain pays)
            m = mpool.tile([P, F], BF16)
            s = mpool.tile([P, 9, F], BF16, tag="mtree")
            for i in range(9):
                nc.vector.tensor_tensor(s[:, i, :], x[:, 2 * i, :],
                                        x[:, 2 * i + 1, :], Alu.max)
            for i in range(4):
                nc.vector.tensor_tensor(s[:, i, :], s[:, 2 * i, :],
                                        s[:, 2 * i + 1, :], Alu.max)
            nc.vector.tensor_tensor(s[:, 0, :], s[:, 0, :], s[:, 1, :], Alu.max)
            nc.vector.tensor_tensor(s[:, 2, :], s[:, 2, :], s[:, 3, :], Alu.max)
            nc.vector.tensor_tensor(s[:, 0, :], s[:, 0, :], s[:, 2, :], Alu.max)
            nc.vector.tensor_tensor(s[:, 8, :], s[:, 8, :], x[:, 18, :], Alu.max)
            nc.vector.tensor_tensor(m[:], s[:, 0, :], s[:, 8, :], Alu.max)

            # pass B: d_c = x_c - m computed by the DMA engines' CCE adder:
            # ship -m to a DRAM scratch, then accumulate-add it into each
            # class slice in place (SWDGE CCE add, bit-exact bf16). Then the
            # exact indicator eq_c = Relu(2^60*d + 1) on ACT with fused
            # p_count accumulation.
            negm = mpool.tile([P, F], BF16, tag="negm")
            nc.vector.tensor_scalar(negm[:], m[:], -1.0, None, Alu.mult)
            nc.sync.dma_start(negm_d[k], negm[:])
            for c in range(C):
                col = acc_t[:, C + c * NCHUNK + k : C + c * NCHUNK + k + 1]
                nc.gpsimd.dma_start(x[:, c, :], negm_d[k], accum_op=Alu.add)
                nc.scalar.activation(x[:, c, :], x[:, c, :], Act.Relu,
                                     bias=1.0, scale=float(2.0 ** 60),
                                     accum_out=col)

            # pass C: (t == c) * eq_c; accum overlap (DVE STT)
            for c in range(C):
                col = acc[:, c * NCHUNK + k : c * NCHUNK + k + 1]
                junk = jpool.tile([P, F], BF16, tag="jc")
                nc.vector.scalar_tensor_tensor(
                    junk[:], tk, float(c), x[:, c, :], Alu.is_equal,
                    Alu.mult, accum_out=col)

        nc.sync.dma_start(out[:, :PC_COLS + N_TC_DVE], acc[:])
        nc.sync.dma_start(out[:, PC_COLS + N_TC_DVE:], acc_t[:])

    nc.compile()
    return nc


_NC_CACHE = None


def _get_nc():
    global _NC_CACHE
    if _NC_CACHE is None:
        _NC_CACHE = build_program()
    return _NC_CACHE


def kernel(pred: np.ndarray, target: np.ndarray, _want_results=False):
    """pred [8,19,512,512] f32, target [8,512,512] int64 -> scalar f32 loss."""
    nc = _get_nc()
    in_maps = []
    for i in range(N_CORES):
        in_maps.append({
            "pred": np.ascontiguousarray(pred[i].reshape(C, PIX)).astype(bfloat16),
            "t": target[i].reshape(PIX).astype(bfloat16),
        })
    res = run_bass_kernel_spmd(nc, in_maps, core_ids=list(range(N_CORES)))
    outs = [r["out"] for r in res.results]  # each [128, NCOL]
    agg = np.sum(np.stack(outs).astype(np.float64), axis=(0, 1))  # [NCOL]
    ov = agg[:PC_COLS].reshape(C, NCHUNK).sum(axis=1)
    w_dve = agg[PC_COLS:PC_COLS + N_TC_DVE]          # moments j=C-8..C-1, max-form
    w_act = agg[PC_COLS + N_TC_DVE:PC_COLS + N_TC_DVE + C]  # Relu-form, j<C-8 valid
    # Relu(t-c) = max(t,c) - c, summed over N_t elems: W_relu = W_max - c*N_t.
    # Both forms yield identical first differences, so just convert the DVE
    # max-moments into Relu-moment form before the shared decode.
    n_t = np.float64(N_CORES * PIX)
    w = w_act.copy()
    for i in range(N_TC_DVE):
        j = C - N_TC_DVE + i
        w[j] = w_dve[i] - (j - 1) * n_t
    pc = agg[PC_COLS + N_TC_DVE + C:].reshape(C, NCHUNK).sum(axis=1)
    wfull = np.concatenate([w, [0.0]])  # append W_18 = 0
    cum_ge = wfull[:-1] - wfull[1:]     # #{t >= c+1} for c = -1..17 -> #{t>=0..18}
    tc = cum_ge.copy()
    tc[:-1] -= cum_ge[1:]               # n_c = #{t>=c} - #{t>=c+1}
    pc32 = pc.astype(np.float32)
    ov32 = ov.astype(np.float32)
    tc32 = tc.astype(np.float32)
    dice = np.float32(2.0) * ov32 / (pc32 + tc32 + np.float32(1.0))
    loss = np.float32(1.0) - dice.sum(dtype=np.float32) / np.float32(8 * C)
    if _want_results:
        return np.float32(loss), res
    return np.float32(loss)


# revision 31
# speedup vs baseline: 1.0875x; 1.0875x over previous
"""DiceLoss Trainium2 kernel (8-core data-parallel SPMD, bf16).

Math (equivalent to the reference):
  softmax over channels is monotone, so pred_cls = argmax_c pred[:, c].
  p_counts[c] = #{pixels: argmax == c}
  t_counts[c] = #{pixels: target == c}
  overlap[c]  = #{pixels: argmax == c and target == c}
  dice = 2*overlap / (p_counts + t_counts + 1);  loss = 1 - dice.sum()/(N*C)

Sharding: batch dim across the 8 NeuronCores (one image per core). Inputs
are shipped as bf16 (halves HBM traffic and doubles DVE tensor_tensor
throughput); the only deviation from the f32 reference is argmax flips on
near-ties after bf16 rounding (~0.4% of pixels), worth ~2.5e-5 relative
error on the final scalar. All counting below is exact.

Device algorithm per core (pred shard [19, 512*512], t shard, 2 pixel
chunks, staged sub-DMAs so compute starts while the chunk streams in):
  pass A (DVE):  per-pixel max m over the 19 channels, pairwise max tree.
  pass B (DVE+ACT): d_c = x_c - m on DVE (2x-mode tensor_tensor), then the
      Activation engine computes the *exact* indicator
      eq_c = Relu(2^60 * d_c + 1)  (d==0 -> 1; any nonzero bf16 d has
      |2^60*d| >> 1 so Relu clamps to 0), with fused accum_out giving the
      per-partition p_count partial for free.
  pass C (DVE): scalar_tensor_tensor (t == c) * eq_c with fused accum ->
      overlap partials.
  t_counts (ACT): cumulative ReLU moments W_j = sum_i Relu(t_i - (j-1));
      the host recovers counts by second differencing. Exact: per-partition
      sums stay under 2^24.
Partials are [128, col] f32 integers -> DMA'd out, summed exactly on the
host across partitions/cores, and combined into the final f32 scalar.

Measured on trn2: ~113 us HW exec (f32 memory roofline for this problem is
~56 us/core; DVE and ACT both run ~86% busy - the kernel is compute-bound
on the two elementwise-capable engines; GPSIMD tensor ops are rejected by
this walrus pipeline and the PE has no role in argmax/counting).
"""

import sys

for _p in ("/opt/trn_rl_repo",):
    if _p not in sys.path:
        sys.path.insert(0, _p)

from contextlib import ExitStack

import numpy as np
from ml_dtypes import bfloat16

import concourse.bass as bass
import concourse.bacc as bacc
import concourse.mybir as mybir
import concourse.tile as tile
from concourse.bass_utils import run_bass_kernel_spmd

# Problem constants (hardcoded; kernel.py must be self-contained).
N_CORES = 8
C = 19
H = W = 512
PIX = H * W  # pixels per core = 262144
P = 128  # SBUF partitions
FTOT = PIX // P  # 2048 free elems per partition
NCHUNK = 2
F = FTOT // NCHUNK  # 512 pixels per partition per chunk

FP32 = mybir.dt.float32
BF16 = mybir.dt.bfloat16
Alu = mybir.AluOpType
Act = mybir.ActivationFunctionType

# Output accumulator layout: [128, NCOL]
#   p_counts: col  (c*NCHUNK + k)            for c in 0..18, k chunk
#   overlap:  col  PC_COLS + (c*NCHUNK + k)
#   t_counts: col  2*PC_COLS + c
PC_COLS = C * NCHUNK
N_TC_DVE = 1
NCOL = 2 * PC_COLS + C + N_TC_DVE



def build_program():
    nc = bacc.Bacc("TRN2", target_bir_lowering=False, debug=False,
                   num_devices=N_CORES)
    pred = nc.dram_tensor("pred", [C, PIX], BF16, kind="ExternalInput").ap()
    tin = nc.dram_tensor("t", [PIX], BF16, kind="ExternalInput").ap()
    out = nc.dram_tensor("out", [P, NCOL], FP32, kind="ExternalOutput").ap()

    # DRAM views: chunk k, partition p, class c, free f
    pred_r = pred.rearrange("c (k p f) -> k p c f", k=NCHUNK, p=P, f=F)
    t_r = tin.rearrange("(k p f) -> p k f", k=NCHUNK, p=P, f=F)

    with tile.TileContext(nc) as tc, ExitStack() as ctx:
        xpool = ctx.enter_context(tc.tile_pool(name="x", bufs=2))
        mpool = ctx.enter_context(tc.tile_pool(name="m", bufs=2))
        jpool = ctx.enter_context(tc.tile_pool(name="junk", bufs=4))
        tpool = ctx.enter_context(tc.tile_pool(name="t", bufs=1))
        apool = ctx.enter_context(tc.tile_pool(name="acc", bufs=1))
        spool = ctx.enter_context(tc.tile_pool(name="scr", bufs=2))

        acc = apool.tile([P, PC_COLS + N_TC_DVE], FP32)  # DVE accums
        acc_t = apool.tile([P, C + PC_COLS], FP32)  # ACT-written accums
        nc.gpsimd.memset(acc_t[:, :C], 0.0)  # W slot j=18 lives in acc

        # t resident for the whole kernel: [128, (k f)]
        t_all = tpool.tile([P, NCHUNK * F], BF16)
        nc.sync.dma_start(
            t_all[:].rearrange("p (k f) -> p k f", k=NCHUNK, f=F), t_r)

        # per-class bias columns: cbias[:, j] = -(j - 1) = 1, 0, -1, ..., -17
        cbias_i = apool.tile([P, C], mybir.dt.int32)
        nc.gpsimd.iota(cbias_i[:], [[1, C]], channel_multiplier=0)
        cbias = apool.tile([P, C], FP32)
        nc.scalar.activation(cbias[:], cbias_i[:], Act.Copy, scale=-1.0,
                             bias=1.0)

        # ---- t_counts on ACT via cumulative ReLU moments ----
        # W_c = sum_i Relu(t_i - c) for c = -1..17 (W_18 = 0); host recovers
        # n_c = (W_{c-1}-W_c) - (W_c-W_{c+1}) exactly (integer partial sums
        # stay under 2^24 per partition).
        for j in range(C):
            if j >= C - N_TC_DVE:
                # W_j = sum max(t, j-1) on DVE (4x tensor_scalar); host
                # converts to Relu-moment form by subtracting (j-1)*N.
                uj = spool.tile([P, NCHUNK * F], BF16, tag="dvu")
                nc.vector.tensor_scalar(
                    uj[:], t_all[:], float(j - 1), 0.0, Alu.max, Alu.add,
                    accum_out=acc[:, PC_COLS + (j - (C - N_TC_DVE)):
                                  PC_COLS + (j - (C - N_TC_DVE)) + 1])
            else:
                u = spool.tile([P, NCHUNK * F], BF16, tag="actu")
                nc.scalar.activation(u[:], t_all[:], Act.Relu,
                                     bias=cbias[:, j:j + 1],
                                     accum_out=acc_t[:, j:j + 1])

        # ---- main per-chunk passes ----
        for k in range(NCHUNK):
            x = xpool.tile([P, C, F], BF16)
            # staged sub-DMAs: pass A can start on classes 0-1 while the
            # rest of the chunk is still in flight
            for lo_c, hi_c in ((0, 2), (2, 8), (8, 14), (14, C)):
                nc.sync.dma_start(x[:, lo_c:hi_c, :], pred_r[k, :, lo_c:hi_c, :])
            tk = t_all[:, k * F:(k + 1) * F]

            # pass A: pairwise max tree (independent ops per level avoid
            # the RAW pipeline stalls a serial chain pays)
            m = mpool.tile([P, F], BF16)
            s = mpool.tile([P, 9, F], BF16, tag="mtree")
            for i in range(9):
                nc.vector.tensor_tensor(s[:, i, :], x[:, 2 * i, :],
                                        x[:, 2 * i + 1, :], Alu.max)
            for i in range(4):
                nc.vector.tensor_tensor(s[:, i, :], s[:, 2 * i, :],
                                        s[:, 2 * i + 1, :], Alu.max)
            nc.vector.tensor_tensor(s[:, 0, :], s[:, 0, :], s[:, 1, :], Alu.max)
            nc.vector.tensor_tensor(s[:, 2, :], s[:, 2, :], s[:, 3, :], Alu.max)
            nc.vector.tensor_tensor(s[:, 0, :], s[:, 0, :], s[:, 2, :], Alu.max)
            nc.vector.tensor_tensor(s[:, 8, :], s[:, 8, :], x[:, 18, :], Alu.max)
            nc.vector.tensor_tensor(m[:], s[:, 0, :], s[:, 8, :], Alu.max)

            # pass B: d_c = x_c - m on DVE (fast TT), then the exact
            # indicator eq_c = Relu(2^60*d + 1) on ACT with fused p_count
            # accumulation. d==0 iff x_c==m; any nonzero bf16 d has
            # |d| >= 2^-133, so 2^60*d <= -1 kills the Relu exactly.
            for c in range(C):
                col = acc_t[:, C + c * NCHUNK + k : C + c * NCHUNK + k + 1]
                nc.vector.tensor_tensor(
                    x[:, c, :], x[:, c, :], m[:], Alu.subtract)
                nc.scalar.activation(x[:, c, :], x[:, c, :], Act.Relu,
                                     bias=1.0, scale=float(2.0 ** 60),
                                     accum_out=col)

            # pass C: (t == c) * eq_c; accum overlap (DVE STT)
            for c in range(C):
                col = acc[:, c * NCHUNK + k : c * NCHUNK + k + 1]
                junk = jpool.tile([P, F], BF16, tag="jc")
                nc.vector.scalar_tensor_tensor(
                    junk[:], tk, float(c), x[:, c, :], Alu.is_equal,
                    Alu.mult, accum_out=col)

        nc.sync.dma_start(out[:, :PC_COLS + N_TC_DVE], acc[:])
        nc.sync.dma_start(out[:, PC_COLS + N_TC_DVE:], acc_t[:])

    nc.compile()
    return nc


_NC_CACHE = None


def _get_nc():
    global _NC_CACHE
    if _NC_CACHE is None:
        _NC_CACHE = build_program()
    return _NC_CACHE


def kernel(pred: np.ndarray, target: np.ndarray, _want_results=False):
    """pred [8,19,512,512] f32, target [8,512,512] int64 -> scalar f32 loss."""
    nc = _get_nc()
    in_maps = []
    for i in range(N_CORES):
        in_maps.append({
            "pred": np.ascontiguousarray(pred[i].reshape(C, PIX)).astype(bfloat16),
            "t": target[i].reshape(PIX).astype(bfloat16),
        })
    res = run_bass_kernel_spmd(nc, in_maps, core_ids=list(range(N_CORES)))
    outs = [r["out"] for r in res.results]  # each [128, NCOL]
    agg = np.sum(np.stack(outs).astype(np.float64), axis=(0, 1))  # [NCOL]
    ov = agg[:PC_COLS].reshape(C, NCHUNK).sum(axis=1)
    w_dve = agg[PC_COLS:PC_COLS + N_TC_DVE]
    w = agg[PC_COLS + N_TC_DVE:PC_COLS + N_TC_DVE + C].copy()
    for i in range(N_TC_DVE):
        j = C - N_TC_DVE + i
        w[j] = w_dve[i] - (j - 1) * np.float64(N_CORES * PIX)
    pc = agg[PC_COLS + N_TC_DVE + C:].reshape(C, NCHUNK).sum(axis=1)
    wfull = np.concatenate([w, [0.0]])  # append W_18 = 0
    cum_ge = wfull[:-1] - wfull[1:]     # #{t >= c+1} for c = -1..17 -> #{t>=0..18}
    tc = cum_ge.copy()
    tc[:-1] -= cum_ge[1:]               # n_c = #{t>=c} - #{t>=c+1}
    pc32 = pc.astype(np.float32)
    ov32 = ov.astype(np.float32)
    tc32 = tc.astype(np.float32)
    dice = np.float32(2.0) * ov32 / (pc32 + tc32 + np.float32(1.0))
    loss = np.float32(1.0) - dice.sum(dtype=np.float32) / np.float32(8 * C)
    if _want_results:
        return np.float32(loss), res
    return np.float32(loss)
